# revision 3
# baseline (speedup 1.0000x reference)
"""EuclideanCodebook (VQ) forward on 8 Trainium2 NeuronCores.

Data-parallel over the flattened N = batch*frames axis: each core gets an
8192-row slab of x and the full [1024, 128] codebook.

Per core, per 128-row tile:
  PE:  dist_psum = -||e_k||^2 (C=1 matmul init) + 2*x·e^T (accumulate)
       (the ||x||^2 term of the reference is row-constant -> argmax-invariant)
  DVE: Max8 + max_index directly on PSUM -> argmax index per row
  GPSIMD: indirect DMA gather embed[idx] -> quantize rows

Row mapping per core: n_local = c*2048 + p*16 + j  for chunk c in [0,4),
partition p in [0,128), slot j in [0,16).  This makes the big DMAs (x in,
q out) 128-partition transfers with 8KB contiguous runs per partition.
"""

import numpy as np

import concourse.bacc as bacc
import concourse.bass as bass
import concourse.mybir as mybir
import concourse.tile as tile
from concourse.bass_utils import run_bass_kernel_spmd
from concourse.masks import make_identity

N_CORES = 8
N_FULL = 65536          # 8 * 8192 rows
N_LOC = N_FULL // N_CORES   # 8192 rows per core
K = 1024                # codebook size
D = 128                 # feature dim
P = 128                 # partitions
CHUNKS = 4              # row chunks per core
JROWS = 16              # rows per partition per chunk (CHUNKS*JROWS*P = N_LOC)
NEG_INF = -3.0e38

FP32 = mybir.dt.float32
U32 = mybir.dt.uint32
I32 = mybir.dt.int32

_COMPILED = {}


def _build():
    nc = bacc.Bacc(
        "TRN2", target_bir_lowering=False, debug=False, num_devices=N_CORES
    )
    x_d = nc.dram_tensor("x", [N_LOC, D], FP32, kind="ExternalInput")
    e_d = nc.dram_tensor("e", [K, D], FP32, kind="ExternalInput")
    q_d = nc.dram_tensor("q", [N_LOC, D], FP32, kind="ExternalOutput")
    ind_d = nc.dram_tensor("ind", [P, CHUNKS * JROWS], I32, kind="ExternalOutput")

    with tile.TileContext(nc) as tc:
        with (
            tc.tile_pool(name="const", bufs=1) as const_pool,
            tc.tile_pool(name="setup", bufs=1) as setup_pool,
            tc.tile_pool(name="xin", bufs=2) as x_pool,
            tc.tile_pool(name="xt", bufs=2) as xt_pool,
            tc.tile_pool(name="g", bufs=2) as g_pool,
            tc.tile_pool(name="mx8", bufs=2) as mx8_pool,
            tc.tile_pool(name="idx8", bufs=2) as idx8_pool,
            tc.tile_pool(name="ptp", bufs=2, space="PSUM") as ptp_pool,
            tc.tile_pool(name="pdist", bufs=2, space="PSUM") as pdist_pool,
            tc.tile_pool(name="psetup", bufs=1, space="PSUM") as psetup_pool,
        ):
            # ---- constants -------------------------------------------------
            identity = const_pool.tile([P, P], FP32)
            make_identity(nc, identity[:])
            ones025n = const_pool.tile([P, 1], FP32)
            nc.gpsimd.memset(ones025n[:], -0.25)
            ones1 = const_pool.tile([1, P], FP32)
            nc.gpsimd.memset(ones1[:], 1.0)
            idxall = const_pool.tile([P, CHUNKS * JROWS], I32)

            # ---- setup: embT2 = 2*embed^T [D, K]; e2neg = -||e_k||^2 [1,K] -
            emb_nat = setup_pool.tile([P, K], FP32)  # [p, (t d)] = e[t*128+p, d]
            nc.sync.dma_start(
                out=emb_nat[:].rearrange("p (t d) -> p t d", t=K // P),
                in_=e_d.ap().rearrange("(t p) d -> p t d", p=P),
            )
            embT2 = const_pool.tile([P, K], FP32)  # [d, k] = 2*e[k, d]
            for t in range(K // P):
                tp = ptp_pool.tile([P, P], FP32, space="PSUM", tag="tp")
                nc.tensor.transpose(
                    out=tp[:],
                    in_=emb_nat[:, t * P : (t + 1) * P],
                    identity=identity[:],
                )
                nc.vector.tensor_scalar_mul(
                    embT2[:, t * P : (t + 1) * P], tp[:], 2.0
                )
            # e2neg[0, k] = -sum_d (2 e[k,d])^2 / 4 = -||e_k||^2
            sq = setup_pool.tile([P, K], FP32)
            nc.vector.tensor_mul(sq[:], embT2[:], embT2[:])
            e2row_ps = psetup_pool.tile([P, K], FP32, space="PSUM", tag="pst")
            for h in range(2):
                sl = slice(h * 512, (h + 1) * 512)
                nc.tensor.matmul(
                    out=e2row_ps[0:1, sl],
                    lhsT=ones025n[:],
                    rhs=sq[:, sl],
                    start=True,
                    stop=True,
                )
            e2neg = const_pool.tile([1, K], FP32)
            nc.vector.tensor_copy(e2neg[:], e2row_ps[0:1, :])

            # ---- main loop -------------------------------------------------
            for c in range(CHUNKS):
                rows = slice(c * P * JROWS, (c + 1) * P * JROWS)
                xc = x_pool.tile([P, JROWS * D], FP32)
                nc.sync.dma_start(
                    out=xc[:].rearrange("p (j d) -> p j d", j=JROWS),
                    in_=x_d.ap()[rows, :].rearrange("(p j) d -> p j d", p=P),
                )
                gth = g_pool.tile([P, JROWS * D], FP32)
                for j in range(JROWS):
                    col = c * JROWS + j
                    xt_ps = ptp_pool.tile([P, P], FP32, space="PSUM", tag="tp")
                    nc.tensor.transpose(
                        out=xt_ps[:],
                        in_=xc[:, j * D : (j + 1) * D],
                        identity=identity[:],
                    )
                    xt = xt_pool.tile([P, P], FP32)
                    nc.scalar.activation(
                        xt[:], xt_ps[:], mybir.ActivationFunctionType.Copy
                    )
                    dist_ps = pdist_pool.tile([P, K], FP32, space="PSUM", tag="pd")
                    for h in range(2):
                        sl = slice(h * 512, (h + 1) * 512)
                        nc.tensor.matmul(
                            out=dist_ps[:, sl],
                            lhsT=ones1[:],
                            rhs=e2neg[:, sl],
                            start=True,
                            stop=False,
                        )
                        nc.tensor.matmul(
                            out=dist_ps[:, sl],
                            lhsT=xt[:],
                            rhs=embT2[:, sl],
                            start=False,
                            stop=True,
                        )
                    mx8 = mx8_pool.tile([P, 8], FP32)
                    nc.vector.max(mx8[:], dist_ps[:])
                    idx8 = idx8_pool.tile([P, 8], U32)
                    nc.vector.max_index(idx8[:], mx8[:], dist_ps[:])
                    nc.vector.tensor_copy(
                        idxall[:, col : col + 1], idx8[:, 0:1]
                    )
                    # dequantize gather for this tile's 128 rows
                    nc.gpsimd.indirect_dma_start(
                        out=gth[:, j * D : (j + 1) * D],
                        out_offset=None,
                        in_=e_d.ap(),
                        in_offset=bass.IndirectOffsetOnAxis(
                            ap=idxall[:, col : col + 1], axis=0
                        ),
                    )
                nc.sync.dma_start(
                    out=q_d.ap()[rows, :].rearrange("(p j) d -> p j d", p=P),
                    in_=gth[:].rearrange("p (j d) -> p j d", j=JROWS),
                )
            nc.sync.dma_start(out=ind_d.ap(), in_=idxall[:])

    nc.compile()
    return nc


def kernel(x: np.ndarray, embed: np.ndarray):
    x = np.ascontiguousarray(np.asarray(x, dtype=np.float32))
    embed = np.ascontiguousarray(np.asarray(embed, dtype=np.float32))
    lead_shape = x.shape[:-1]
    xf = x.reshape(-1, D)
    assert xf.shape == (N_FULL, D) and embed.shape == (K, D)

    if "nc" not in _COMPILED:
        _COMPILED["nc"] = _build()
    nc = _COMPILED["nc"]

    in_maps = [
        {"x": xf[i * N_LOC : (i + 1) * N_LOC], "e": embed} for i in range(N_CORES)
    ]
    res = run_bass_kernel_spmd(nc, in_maps, list(range(N_CORES)))

    q = np.empty((N_FULL, D), dtype=np.float32)
    ind = np.empty((N_FULL,), dtype=np.int32)
    for i in range(N_CORES):
        q[i * N_LOC : (i + 1) * N_LOC] = res.results[i]["q"]
        # ind_d is [P, CHUNKS*JROWS]; local row n = c*2048 + p*16 + j maps to
        # column c*16 + j -> reshape/transpose back to n-order.
        ia = res.results[i]["ind"].reshape(P, CHUNKS, JROWS)
        ind[i * N_LOC : (i + 1) * N_LOC] = (
            ia.transpose(1, 0, 2).reshape(N_LOC).astype(np.int32)
        )

    quantize = q.reshape(*lead_shape, D)
    embed_ind = ind.reshape(*lead_shape)
    return quantize, embed_ind


# revision 17
# speedup vs baseline: 21.5098x; 21.5098x over previous
"""EuclideanCodebook (VQ) forward on 8 Trainium2 NeuronCores.

Data-parallel over the flattened N = batch*frames axis: each core gets an
8192-row slab of x and the full [1024, 128] codebook.

Host-side prep is layout-only (transposes for DMA/matmul efficiency); all
arithmetic runs on device:

  PE:  dist = 2*x·e^T (fp32); for 3 of every 5 tiles PE also seeds the
       PSUM with -||e_k||^2 via a C=1 matmul (the ||x||^2 term of the
       reference is row-constant -> argmax-invariant)
  DVE: for the other 2 of 5 tiles, adds -||e_k||^2 while copying PSUM ->
       SBUF; Max8 + MaxIndex give the exact fp32 argmax per row.  The
       3:5 split balances PE and DVE busy time.
  GPSIMD: indirect DMA gather embed[idx] -> quantize rows

Device layouts (per core): row n = t*128 + p for tile t in [0,64),
partition p in [0,128).  x arrives transposed ([128 d, 8192 n]) so every
tile's lhsT is a direct SBUF slice; q leaves as [128 p, 64 t, 128 d] and
the host permutes back to row order.
"""

import numpy as np

import concourse.bacc as bacc
import concourse.bass as bass
import concourse.mybir as mybir
import concourse.tile as tile
from concourse.bass_utils import run_bass_kernel_spmd

N_CORES = 8
N_FULL = 65536          # 8 * 8192 rows
N_LOC = N_FULL // N_CORES   # 8192 rows per core
K = 1024                # codebook size
D = 128                 # feature dim
P = 128                 # partitions
TILES = N_LOC // P      # 64 row-tiles per core
CHUNK_SIZES = (4, 12, 16, 16, 12, 4)   # tiles per x/q DMA chunk

FP32 = mybir.dt.float32
U32 = mybir.dt.uint32
I32 = mybir.dt.int32

_COMPILED = {}


def _build(reps=1):
    nc = bacc.Bacc(
        "TRN2", target_bir_lowering=False, debug=False, num_devices=N_CORES
    )
    xt_d = nc.dram_tensor("xt", [P, N_LOC], FP32, kind="ExternalInput")
    eT_d = nc.dram_tensor("eT", [P, K], FP32, kind="ExternalInput")
    e_d = nc.dram_tensor("e", [K, D], FP32, kind="ExternalInput")
    q_d = nc.dram_tensor("q", [P, TILES * D], FP32, kind="ExternalOutput")
    # one 8-wide uint32 slot per row-tile (MaxIndex writes all 8 lanes);
    # host keeps lane 0 of each slot.
    ind_d = nc.dram_tensor("ind", [P, TILES * 8], I32, kind="ExternalOutput")

    with tile.TileContext(nc) as tc:
        with (
            tc.tile_pool(name="const", bufs=1) as const_pool,
            tc.tile_pool(name="xin", bufs=3) as x_pool,
            tc.tile_pool(name="g", bufs=2) as g_pool,
            tc.tile_pool(name="mx8", bufs=2) as mx8_pool,
            tc.tile_pool(name="s", bufs=3) as s_pool,
            tc.tile_pool(name="pdist", bufs=4, space="PSUM") as pdist_pool,
        ):
            # ---- constants / setup ----------------------------------------
            negq = const_pool.tile([P, P], FP32)   # all -0.25
            nc.gpsimd.memset(negq[:], -0.25)
            ones1 = const_pool.tile([1, P], FP32)
            nc.gpsimd.memset(ones1[:], 1.0)
            idxbig = const_pool.tile([P, TILES * 8], U32)

            embT2 = const_pool.tile([P, K], FP32)  # [d, k] = 2*e[k, d]
            eTs = const_pool.tile([P, K], FP32)
            nc.scalar.dma_start(eTs[:], eT_d.ap())
            nc.vector.tensor_scalar_mul(embT2[:], eTs[:], 2.0)
            # sq = (2 e)^2 on ACT; e2negbc[m, k] = -0.25*sum_d sq = -||e_k||^2
            sq = const_pool.tile([P, K], FP32)
            nc.scalar.activation(
                sq[:], embT2[:], mybir.ActivationFunctionType.Square
            )
            e2bc_ps = pdist_pool.tile([P, K], FP32, space="PSUM", tag="pd")
            for h in range(2):
                sl = slice(h * 512, (h + 1) * 512)
                nc.tensor.matmul(
                    out=e2bc_ps[:, sl],
                    lhsT=negq[:],
                    rhs=sq[:, sl],
                    start=True,
                    stop=True,
                )
            e2negbc = const_pool.tile([P, K], FP32)
            nc.vector.tensor_copy(e2negbc[:], e2bc_ps[:])
            e2neg = e2negbc[0:1, :]   # row view for the C=1 PE seed

            # ---- main loop (reps>1 only for steady-state timing probes) ---
            for _rep in range(reps):
                t0 = 0
                for tpc in CHUNK_SIZES:
                    xc = x_pool.tile([P, 16 * P], FP32, tag="xc")
                    nc.sync.dma_start(
                        xc[:, : tpc * P], xt_d.ap()[:, t0 * P : (t0 + tpc) * P]
                    )
                    gth = g_pool.tile([P, 16 * D], FP32, tag="gth")
                    for j in range(tpc):
                        t = t0 + j
                        on_pe = (t % 5) < 3
                        dist_ps = pdist_pool.tile(
                            [P, K], FP32, space="PSUM", tag="pd"
                        )
                        if on_pe:
                            # PE seeds -||e_k||^2 (C=1), accumulates 2x·e;
                            # DVE scans straight from PSUM.
                            for h in range(2):
                                sl = slice(h * 512, (h + 1) * 512)
                                nc.tensor.matmul(
                                    out=dist_ps[:, sl],
                                    lhsT=ones1[:],
                                    rhs=e2neg[:, sl],
                                    start=True,
                                    stop=False,
                                )
                                nc.tensor.matmul(
                                    out=dist_ps[:, sl],
                                    lhsT=xc[:, j * P : (j + 1) * P],
                                    rhs=embT2[:, sl],
                                    start=False,
                                    stop=True,
                                )
                            scan_src = dist_ps[:]
                        else:
                            # PE computes raw 2x·e; DVE folds in -||e_k||^2
                            # while copying to SBUF, then scans there.
                            for h in range(2):
                                sl = slice(h * 512, (h + 1) * 512)
                                nc.tensor.matmul(
                                    out=dist_ps[:, sl],
                                    lhsT=xc[:, j * P : (j + 1) * P],
                                    rhs=embT2[:, sl],
                                    start=True,
                                    stop=True,
                                )
                            s = s_pool.tile([P, K], FP32, tag="s")
                            nc.vector.tensor_add(s[:], dist_ps[:], e2negbc[:])
                            scan_src = s[:]
                        mx8 = mx8_pool.tile([P, 8], FP32)
                        nc.vector.max(mx8[:], scan_src)
                        nc.vector.max_index(
                            idxbig[:, t * 8 : (t + 1) * 8], mx8[:], scan_src
                        )
                        # dequantize gather for this tile's 128 rows
                        nc.gpsimd.indirect_dma_start(
                            out=gth[:, j * D : (j + 1) * D],
                            out_offset=None,
                            in_=e_d.ap(),
                            in_offset=bass.IndirectOffsetOnAxis(
                                ap=idxbig[:, t * 8 : t * 8 + 1], axis=0
                            ),
                        )
                    nc.sync.dma_start(
                        q_d.ap()[:, t0 * D : (t0 + tpc) * D], gth[:, : tpc * D]
                    )
                    nc.scalar.dma_start(
                        out=ind_d.ap()[:, t0 * 8 : (t0 + tpc) * 8],
                        in_=idxbig[:, t0 * 8 : (t0 + tpc) * 8].bitcast(I32),
                    )
                    t0 += tpc

    nc.compile()
    return nc


def kernel(x: np.ndarray, embed: np.ndarray):
    x = np.asarray(x, dtype=np.float32)
    embed = np.ascontiguousarray(np.asarray(embed, dtype=np.float32))
    lead_shape = x.shape[:-1]
    xf = x.reshape(-1, D)
    assert xf.shape == (N_FULL, D) and embed.shape == (K, D)
    xT = np.ascontiguousarray(xf.T)          # [128, 65536], layout-only
    eT = np.ascontiguousarray(embed.T)       # [128, 1024], layout-only

    if "nc" not in _COMPILED:
        _COMPILED["nc"] = _build()
    nc = _COMPILED["nc"]

    in_maps = [
        {
            "xt": np.ascontiguousarray(xT[:, i * N_LOC : (i + 1) * N_LOC]),
            "eT": eT,
            "e": embed,
        }
        for i in range(N_CORES)
    ]
    res = run_bass_kernel_spmd(nc, in_maps, list(range(N_CORES)))

    q = np.empty((N_FULL, D), dtype=np.float32)
    ind = np.empty((N_FULL,), dtype=np.int32)
    for i in range(N_CORES):
        # q device layout [p, t, d] -> row n = t*128 + p
        qi = res.results[i]["q"].reshape(P, TILES, D)
        q[i * N_LOC : (i + 1) * N_LOC] = qi.transpose(1, 0, 2).reshape(N_LOC, D)
        # ind device layout [p, t*8] (lane 0 of each 8-wide slot)
        ia = res.results[i]["ind"].reshape(P, TILES, 8)[:, :, 0]
        ind[i * N_LOC : (i + 1) * N_LOC] = ia.T.reshape(N_LOC).astype(np.int32)

    quantize = q.reshape(*lead_shape, D)
    embed_ind = ind.reshape(*lead_shape)
    return quantize, embed_ind
